# revision 12
# baseline (speedup 1.0000x reference)
"""Cost-volume kernel for Trainium2 (8 NeuronCores, Bass).

cost[b, i, h, w] = mean_c f1[b,c,h,w] * f2[b,c,h,w-i]  (0 where w < i)

Host pre-scales inputs by 2^-3.5 (product carries the 1/128 mean scale),
casts to fp16, and reverses f2 along w (f2r[c,v] = f2[c,255-v]).  Device
computes, per (b, h) plane with C=128 on partitions, the banded gram
H2[w, v] = sum_c f1[c,w] * f2r[c,v] = gram[w, 255-v]:

  tile A: w in [0,128),   v in [128,256) -> 128-col matmul (PE fp16)
  tile B: w in [128,256), v in [0,192)   -> 192-col matmul

out[i, w] = H2[w, 255-w+i] -- at fixed w a contiguous 64-run in v with
run-start DESCENDING in w (the DMA-exact anti-diagonal form; ascending
flat strides >= partition pitch corrupt at 4-partition boundaries).
PSUM -> SBUF fp16 copy (one op per pair, ACT/DVE parity split), then ONE
anti-diagonal DMA per PAIR (src steps [767, 192, 1]) shears the band
straight to HBM as out[pair, k, d, j]: plane = 2q + d//2,
w = 128*(d%2) + k, i = j.  The v>=256 region (i > w) is never computed;
the host zeroes that triangle and casts fp32.

HWDGE dma_start costs ~0.6us + ~1ns/descriptor on the issuing
sequencer, so issue work is spread over three rings:
  Pool ring (gpsimd, SWDGE): f1 + f2r OCT loads (8 planes, 512KB, 4KB
      descriptors -> line rate), starting under the framework preamble
  SP ring (sync):    even-pair shear-stores
  ACT ring (scalar): odd-pair shear-stores (+ even-pair HC copies)

Sharding: 8 cores x 16 H-rows (data-parallel over B*H planes, 64 planes/core).
"""
import numpy as np

import concourse.bass as bass
import concourse.mybir as mybir
from concourse.bass_utils import run_bass_kernel_spmd

B, C, H, W = 4, 128, 128, 256
L = 64
NCORES = 8
HS = H // NCORES          # 16 h-rows per core
NPL = B * HS              # 64 planes per core
NPR = NPL // 2            # 32 pairs per core
NQ = NPR // 2             # 16 quads per core
NO = NPL // 8             # 8 octs per core

# stage lags in pair-iterations.
LAG_MM = 5
LAG_HC = 6
LAG_SH = 8
NIT = NPR + LAG_SH + 1

NBO = 8           # F1/F2R oct buffers -- ALL octs resident (8MB SBUF)
NHC = 7           # HC pair buffers
NPH = 4           # PSUM pair slots (2 banks each = all 8 banks)

F32 = mybir.dt.float32
F16 = mybir.dt.float16


def _build(nc_holder={}):
    if "nc" in nc_holder:
        return nc_holder["nc"]
    nc = bass.Bass()
    f1 = nc.dram_tensor("f1", [B, C, HS, W], F16, kind="ExternalInput")
    f2r = nc.dram_tensor("f2r", [B, C, HS, W], F16, kind="ExternalInput")
    out = nc.dram_tensor("out", [NPR, 128, 4, L], F16, kind="ExternalOutput")

    from contextlib import ExitStack
    ctx = ExitStack()
    sem = lambda n: ctx.enter_context(nc.semaphore(n))
    sbuf = lambda n, s, dt: ctx.enter_context(nc.sbuf_tensor(n, s, dt))
    psum = lambda n, s: ctx.enter_context(nc.psum_tensor(n, s, F32))

    sF1 = [sem(f"sF1_{k}") for k in range(NBO)]
    sF2 = [sem(f"sF2_{k}") for k in range(NBO)]
    sSh = [sem(f"sSh_{k}") for k in range(NHC)]
    cM = sem("cM")     # gram mms, +4/pair
    cHe = sem("cHe")   # HC copy even pairs (ACT), +1
    cHo = sem("cHo")   # HC copy odd pairs (DVE), +1

    F1O = [sbuf(f"F1O_{k}", [128, 2048], F16) for k in range(NBO)]
    F2O = [sbuf(f"F2O_{k}", [128, 2048], F16) for k in range(NBO)]
    HC = [sbuf(f"HC_{k}", [128, 768], F16) for k in range(NHC)]
    Hp = [psum(f"Hp_{k}", [128, 1024]) for k in range(NPH)]

    uses = lambda t, n: 16 * (t // n + 1)

    def in_oct(tensor, o):
        b, hl = (8 * o) // HS, (8 * o) % HS
        return bass.AP(tensor, (b * C * HS + hl) * W, [[HS * W, 128], [W, 8], [1, W]])

    def load_oct(engine, o):
        engine.dma_start(F1O[o % NBO][:, :], in_oct(f1, o)).then_inc(sF1[o % NBO], 16)
        engine.dma_start(F2O[o % NBO][:, :], in_oct(f2r, o)).then_inc(sF2[o % NBO], 16)

    def wait_hc(engine, q):
        if q % 2 == 0:
            engine.wait_ge(cHe, q // 2 + 1)
        else:
            engine.wait_ge(cHo, q // 2 + 1)

    def hc_copy(engine, q):
        # HC(q) <- fp16(Hp(q)), both planes, 384 cols each
        # (cols [128,192) per plane are stale tile-A tail -- masked on host)
        engine.wait_ge(cM, 4 * (q + 1))
        if q >= NHC:
            qq = q - NHC
            engine.wait_ge(sSh[qq % NHC], uses(qq, NHC))   # HC slot free
        copy_fn = getattr(engine, "tensor_copy", None) or engine.copy
        copy_fn(
            bass.AP(HC[q % NHC], 0, [[768, 128], [384, 2], [1, 384]]),
            bass.AP(Hp[q % NPH], 0, [[1024, 128], [512, 2], [1, 384]]),
        ).then_inc(cHe if q % 2 == 0 else cHo, 1)

    def shear_store(engine, q):
        # fused shear-store: out[q, k, d, j] = H2[w, 255-w+j],
        # w = 128*(d%2)+k, plane 2q+d//2 (anti-diag steps 767)
        wait_hc(engine, q)                               # HC(q) written
        engine.dma_start(
            bass.AP(out, q * 128 * 4 * L, [[256, 128], [64, 4], [1, 64]]),
            bass.AP(HC[q % NHC], 127, [[767, 128], [192, 4], [1, 64]]),
        ).then_inc(sSh[q % NHC], 16)

    with nc.Block() as block:

        @block.gpsimd
        def _(gpsimd):
            for o in range(NO):
                load_oct(gpsimd, o)

        @block.sync
        def _(sync):
            for i in range(NIT):
                q = i - LAG_SH
                if 0 <= q < NPR and q % 2 == 0:
                    shear_store(sync, q)

        @block.scalar
        def _(scalar):
            for i in range(NIT):
                q = i - LAG_HC
                if 0 <= q < NPR and q % 2 == 0:
                    hc_copy(scalar, q)
                q = i - LAG_SH
                if 0 <= q < NPR and q % 2 == 1:
                    shear_store(scalar, q)

        @block.vector
        def _(vector):
            for i in range(NIT):
                q = i - LAG_HC
                if 0 <= q < NPR and q % 2 == 1:
                    hc_copy(vector, q)

        @block.tensor
        def _(tensor):
            for i in range(NIT):
                q = i - LAG_MM
                if 0 <= q < NPR:
                    o = q // 4
                    tensor.wait_ge(sF1[o % NBO], uses(o, NBO))  # F1 oct loaded
                    tensor.wait_ge(sF2[o % NBO], uses(o, NBO))  # F2R oct loaded
                    if q >= NPH:
                        wait_hc(tensor, q - NPH)                # Hp slot free
                    hp = Hp[q % NPH]
                    f1t, f2t = F1O[o % NBO], F2O[o % NBO]
                    for s in range(2):
                        u = 2 * (q % 4) + s                     # plane in oct
                        fo, po = 256 * u, 512 * s
                        tensor.matmul(hp[:, po:po + 128], f1t[:, fo:fo + 128],
                                      f2t[:, fo + 128:fo + 256]).then_inc(cM, 1)
                        tensor.matmul(hp[:, po + 192:po + 384], f1t[:, fo + 128:fo + 256],
                                      f2t[:, fo:fo + 192]).then_inc(cM, 1)

    nc_holder["nc"] = nc
    return nc


def run_sharded(features_1: np.ndarray, features_2: np.ndarray, **spmd_kwargs):
    """Shard over H, run on 8 cores, return (full_output, BassKernelResults)."""
    nc = _build()
    s = np.float32(2.0 ** -3.5)        # s*s = 1/128 (mean over channels)
    f1h = (features_1 * s).astype(np.float16)
    f2h = (features_2[:, :, :, ::-1] * s).astype(np.float16)   # w-reversed
    in_maps = []
    for k in range(NCORES):
        sl = slice(k * HS, (k + 1) * HS)
        in_maps.append({
            "f1": np.ascontiguousarray(f1h[:, :, sl, :]),
            "f2r": np.ascontiguousarray(f2h[:, :, sl, :]),
        })
    res = run_bass_kernel_spmd(nc, in_maps, core_ids=list(range(NCORES)), **spmd_kwargs)
    full = np.empty((B, L, H, W), dtype=np.float32)
    for k in range(NCORES):
        a = np.asarray(res.results[k]["out"]).reshape(NPR, 128, 2, 2, L)
        # (q, k, s, chunk, j): plane 2q+s, w = chunk*128 + k, i = j
        a = a.transpose(0, 2, 4, 3, 1).reshape(B, HS, L, W)
        full[:, :, k * HS:(k + 1) * HS, :] = a.transpose(0, 2, 1, 3).astype(np.float32)
    for i in range(1, L):              # zero the w < i triangle (uncomputed)
        full[:, i, :, :i] = 0.0
    return full, res


def kernel(features_1, features_2, lvls) -> np.ndarray:
    assert int(lvls) == L
    f1 = np.asarray(features_1, dtype=np.float32)
    f2 = np.asarray(features_2, dtype=np.float32)
    full, _ = run_sharded(f1, f2)
    return full


# revision 13
# speedup vs baseline: 1.0010x; 1.0010x over previous
"""Cost-volume kernel for Trainium2 (8 NeuronCores, Bass).

cost[b, i, h, w] = mean_c f1[b,c,h,w] * f2[b,c,h,w-i]  (0 where w < i)

Host pre-scales inputs by 2^-3.5 (product carries the 1/128 mean scale),
casts to fp16, and reverses f2 along w (f2r[c,v] = f2[c,255-v]).  Device
computes, per (b, h) plane with C=128 on partitions, the banded gram
H2[w, v] = sum_c f1[c,w] * f2r[c,v] = gram[w, 255-v]:

  tile A: w in [0,128),   v in [128,256) -> 128-col matmul (PE fp16)
  tile B: w in [128,256), v in [0,192)   -> 192-col matmul

out[i, w] = H2[w, 255-w+i] -- at fixed w a contiguous 64-run in v with
run-start DESCENDING in w (the DMA-exact anti-diagonal form; ascending
flat strides >= partition pitch corrupt at 4-partition boundaries).
PSUM -> SBUF fp16 copy (one op per pair, ACT/DVE parity split), then ONE
anti-diagonal DMA per PAIR (src steps [767, 192, 1]) shears the band
straight to HBM as out[pair, k, d, j]: plane = 2q + d//2,
w = 128*(d%2) + k, i = j.  The v>=256 region (i > w) is never computed;
the host zeroes that triangle and casts fp32.

HWDGE dma_start costs ~0.6us + ~1ns/descriptor on the issuing
sequencer, so issue work is spread over three rings:
  Pool ring (gpsimd, SWDGE): f1 + f2r OCT loads (8 planes, 512KB, 4KB
      descriptors -> line rate), starting under the framework preamble
  SP ring (sync):    even-pair shear-stores
  ACT ring (scalar): odd-pair shear-stores (+ even-pair HC copies)

Sharding: 8 cores x 16 H-rows (data-parallel over B*H planes, 64 planes/core).
"""
import numpy as np

import concourse.bass as bass
import concourse.mybir as mybir
from concourse.bass_utils import run_bass_kernel_spmd

B, C, H, W = 4, 128, 128, 256
L = 64
NCORES = 8
HS = H // NCORES          # 16 h-rows per core
NPL = B * HS              # 64 planes per core
NPR = NPL // 2            # 32 pairs per core
NQ = NPR // 2             # 16 quads per core
NO = NPL // 8             # 8 octs per core

# stage lags in pair-iterations.
LAG_MM = 5
LAG_HC = 6
LAG_SH = 8
NIT = NPR + LAG_SH + 1

NBO = 8           # F1/F2R oct buffers -- ALL octs resident (8MB SBUF)
NHC = 7           # HC pair buffers
NPH = 4           # PSUM pair slots (2 banks each = all 8 banks)

F32 = mybir.dt.float32
F16 = mybir.dt.float16


def _build(nc_holder={}):
    if "nc" in nc_holder:
        return nc_holder["nc"]
    nc = bass.Bass()
    f1 = nc.dram_tensor("f1", [B, C, HS, W], F16, kind="ExternalInput")
    f2r = nc.dram_tensor("f2r", [B, C, HS, W], F16, kind="ExternalInput")
    out = nc.dram_tensor("out", [NPR, 128, 4, L], F16, kind="ExternalOutput")

    from contextlib import ExitStack
    ctx = ExitStack()
    sem = lambda n: ctx.enter_context(nc.semaphore(n))
    sbuf = lambda n, s, dt: ctx.enter_context(nc.sbuf_tensor(n, s, dt))
    psum = lambda n, s: ctx.enter_context(nc.psum_tensor(n, s, F32))

    sF1 = [sem(f"sF1_{k}") for k in range(NBO)]
    sF2 = [sem(f"sF2_{k}") for k in range(NBO)]
    sSh = [sem(f"sSh_{k}") for k in range(NHC)]
    cM = sem("cM")     # gram mms, +4/pair
    cHe = sem("cHe")   # HC copy even pairs (ACT), +1
    cHo = sem("cHo")   # HC copy odd pairs (DVE), +1

    F1O = [sbuf(f"F1O_{k}", [128, 2048], F16) for k in range(NBO)]
    F2O = [sbuf(f"F2O_{k}", [128, 2048], F16) for k in range(NBO)]
    HC = [sbuf(f"HC_{k}", [128, 768], F16) for k in range(NHC)]
    Hp = [psum(f"Hp_{k}", [128, 1024]) for k in range(NPH)]

    uses = lambda t, n: 16 * (t // n + 1)

    def in_oct(tensor, o):
        b, hl = (8 * o) // HS, (8 * o) % HS
        return bass.AP(tensor, (b * C * HS + hl) * W, [[HS * W, 128], [W, 8], [1, W]])

    def load_oct(engine, o):
        engine.dma_start(F1O[o % NBO][:, :], in_oct(f1, o)).then_inc(sF1[o % NBO], 16)
        engine.dma_start(F2O[o % NBO][:, :], in_oct(f2r, o)).then_inc(sF2[o % NBO], 16)

    def wait_hc(engine, q):
        if q % 2 == 0:
            engine.wait_ge(cHe, q // 2 + 1)
        else:
            engine.wait_ge(cHo, q // 2 + 1)

    def hc_copy(engine, q):
        # HC(q) <- fp16(Hp(q)), both planes, 384 cols each
        # (cols [128,192) per plane are stale tile-A tail -- masked on host)
        engine.wait_ge(cM, 4 * (q + 1))
        if q >= NHC:
            qq = q - NHC
            engine.wait_ge(sSh[qq % NHC], uses(qq, NHC))   # HC slot free
        copy_fn = getattr(engine, "tensor_copy", None) or engine.copy
        copy_fn(
            bass.AP(HC[q % NHC], 0, [[768, 128], [384, 2], [1, 384]]),
            bass.AP(Hp[q % NPH], 0, [[1024, 128], [512, 2], [1, 384]]),
        ).then_inc(cHe if q % 2 == 0 else cHo, 1)

    def shear_store(engine, q):
        # fused shear-store: out[q, k, d, j] = H2[w, 255-w+j],
        # w = 128*(d%2)+k, plane 2q+d//2 (anti-diag steps 767)
        wait_hc(engine, q)                               # HC(q) written
        engine.dma_start(
            bass.AP(out, q * 128 * 4 * L, [[256, 128], [64, 4], [1, 64]]),
            bass.AP(HC[q % NHC], 127, [[767, 128], [192, 4], [1, 64]]),
        ).then_inc(sSh[q % NHC], 16)

    with nc.Block() as block:

        @block.gpsimd
        def _(gpsimd):
            for o in range(NO):
                load_oct(gpsimd, o)

        @block.sync
        def _(sync):
            for i in range(NIT):
                q = i - LAG_SH
                if 0 <= q < NPR and q % 2 == 0:
                    shear_store(sync, q)

        @block.scalar
        def _(scalar):
            for i in range(NIT):
                q = i - LAG_HC
                if 0 <= q < NPR and q % 2 == 0:
                    hc_copy(scalar, q)
                q = i - LAG_SH
                if 0 <= q < NPR and q % 2 == 1:
                    shear_store(scalar, q)

        @block.vector
        def _(vector):
            for i in range(NIT):
                q = i - LAG_HC
                if 0 <= q < NPR and q % 2 == 1:
                    hc_copy(vector, q)

        @block.tensor
        def _(tensor):
            # HAM warmup: ~10 dense 512-col matmuls on garbage data during
            # the initial load wait keep the PE activity window busy >3.4us
            # so the clock gate opens (K=8/8) before real matmuls start.
            for _ in range(10):
                tensor.matmul(Hp[3][:, 0:512], F1O[0][:, 0:128], F1O[0][:, 0:512])
            for i in range(NIT):
                q = i - LAG_MM
                if 0 <= q < NPR:
                    o = q // 4
                    if q % 4 == 0:
                        tensor.wait_ge(sF1[o % NBO], uses(o, NBO))  # F1 oct loaded
                        tensor.wait_ge(sF2[o % NBO], uses(o, NBO))  # F2R oct loaded
                    if q >= NPH:
                        wait_hc(tensor, q - NPH)                # Hp slot free
                    hp = Hp[q % NPH]
                    f1t, f2t = F1O[o % NBO], F2O[o % NBO]
                    for s in range(2):
                        u = 2 * (q % 4) + s                     # plane in oct
                        fo, po = 256 * u, 512 * s
                        tensor.matmul(hp[:, po:po + 128], f1t[:, fo:fo + 128],
                                      f2t[:, fo + 128:fo + 256]).then_inc(cM, 1)
                        tensor.matmul(hp[:, po + 192:po + 384], f1t[:, fo + 128:fo + 256],
                                      f2t[:, fo:fo + 192]).then_inc(cM, 1)

    nc_holder["nc"] = nc
    return nc


def run_sharded(features_1: np.ndarray, features_2: np.ndarray, **spmd_kwargs):
    """Shard over H, run on 8 cores, return (full_output, BassKernelResults)."""
    nc = _build()
    s = np.float32(2.0 ** -3.5)        # s*s = 1/128 (mean over channels)
    f1h = (features_1 * s).astype(np.float16)
    f2h = (features_2[:, :, :, ::-1] * s).astype(np.float16)   # w-reversed
    in_maps = []
    for k in range(NCORES):
        sl = slice(k * HS, (k + 1) * HS)
        in_maps.append({
            "f1": np.ascontiguousarray(f1h[:, :, sl, :]),
            "f2r": np.ascontiguousarray(f2h[:, :, sl, :]),
        })
    res = run_bass_kernel_spmd(nc, in_maps, core_ids=list(range(NCORES)), **spmd_kwargs)
    full = np.empty((B, L, H, W), dtype=np.float32)
    for k in range(NCORES):
        a = np.asarray(res.results[k]["out"]).reshape(NPR, 128, 2, 2, L)
        # (q, k, s, chunk, j): plane 2q+s, w = chunk*128 + k, i = j
        a = a.transpose(0, 2, 4, 3, 1).reshape(B, HS, L, W)
        full[:, :, k * HS:(k + 1) * HS, :] = a.transpose(0, 2, 1, 3).astype(np.float32)
    for i in range(1, L):              # zero the w < i triangle (uncomputed)
        full[:, i, :, :i] = 0.0
    return full, res


def kernel(features_1, features_2, lvls) -> np.ndarray:
    assert int(lvls) == L
    f1 = np.asarray(features_1, dtype=np.float32)
    f2 = np.asarray(features_2, dtype=np.float32)
    full, _ = run_sharded(f1, f2)
    return full
